# revision 2
# baseline (speedup 1.0000x reference)
"""Trainium2 Bass kernel v3 for nn_DeepModel_multi_12945031430869.

Computes, for heads h in 0..31:
    y[:, h] = relu(x @ W1[h] + b1[h]) @ W2[h] + b2[h]
    out[:, h*513:(h+1)*513] = [x, y[:, h]]          # [4096, 16416]

Sharding: head-parallel across 8 NeuronCores (4 heads per core).

Algorithm (abs-split): relu(t) = (t + |t|)/2, so
    y = 0.5*x@(W1@W2) + 0.5*sum_f |h_f|*|W2_f|*sign(W2_f) + (b2 + 0.5*b1@W2)
with the tiny linear term computed exactly in bf16 and only the |h| term in
fp8 — sign(h)*eps has half the error variance of relu'(h)*eps, which brings
direct 1-term fp8 from 2.5e-2 under the 2e-2 gate (sim: ~1.75e-2).

Device program (per core):
  - GEMM1 in fp8(e4m3) DoubleRow mode: K=256 per matmul at the cost of a
    K=128 one -> 1024 matmuls instead of 2048. |W2| is folded into W1's
    columns (x8 @ (SW*W1*|w2|) lands in PSUM at scale SW); b1 is dropped from
    the abs term (handled exactly in the linear/constant parts; the cross
    term adds ~3e-3 std, verified in sim).
  - Epilogue per (head, row-tile, 2-bank group): ONE DVE stt
    (|ps| * srow, accum_out) where srow = sign(w2)/(2*SW); per row-tile two
    tiny stt combines fold the group sums and the linear-term PSUM.
  - Linear term: 4 bf16 matmuls per row-tile into a 1-bank PSUM [128, 4]
    (all 4 heads batched as moving columns).
  - Output row-block [128, 513]: x copied from SBUF-resident fp32 x by ACT,
    y written by a tiny ACT op adding b2eff.
PSUM: 3 bufs x 2 banks (GEMM groups) + 2 bufs x 1 bank (linear) = 8 banks.
"""

import numpy as np

N = 4096
D_IN = 512
D_H = 2048
USED = 32
NCORES = 8
HPC = USED // NCORES  # heads per core = 4
KC = 2                # K chunks of 256 (DoubleRow)
KT = 4                # K chunks of 128 (bf16 linear mms)
TT = 4                # f tiles of 512
RT = N // 128         # row tiles = 32
SW = 8192.0

_PROG = None


def _build_program():
    import concourse.tile as tile
    import concourse.mybir as mybir
    from concourse import bacc

    f32 = mybir.dt.float32
    bf16 = mybir.dt.bfloat16
    f8 = mybir.dt.float8e4

    nc = bacc.Bacc("TRN2", target_bir_lowering=False, debug=False)

    x8_d = nc.dram_tensor("x8", [KC, 128, 2 * N], f8, kind="ExternalInput").ap()
    xbf_d = nc.dram_tensor("xbf", [KT, 128, N], bf16, kind="ExternalInput").ap()
    x_d = nc.dram_tensor("x", [RT, 128, D_IN], f32, kind="ExternalInput").ap()
    w8_d = nc.dram_tensor("w8", [HPC, 128, TT * KC * 2 * 512], f8, kind="ExternalInput").ap()
    sr_d = nc.dram_tensor("srow", [128, HPC * D_H], bf16, kind="ExternalInput").ap()
    gt_d = nc.dram_tensor("gt", [128, KT * HPC], bf16, kind="ExternalInput").ap()
    b2_d = nc.dram_tensor("b2r", [128, HPC], f32, kind="ExternalInput").ap()
    out_d = nc.dram_tensor("out", [N, HPC * 513], f32, kind="ExternalOutput").ap()

    absf = mybir.ActivationFunctionType.Abs
    mult = mybir.AluOpType.mult
    add = mybir.AluOpType.add
    ident = mybir.ActivationFunctionType.Identity
    copyf = mybir.ActivationFunctionType.Copy
    DR = mybir.MatmulPerfMode.DoubleRow

    with tile.TileContext(nc) as tc:
        with tc.tile_pool(name="x8p", bufs=1) as x8p, \
             tc.tile_pool(name="xbp", bufs=1) as xbp, \
             tc.tile_pool(name="xsp", bufs=1) as xsp, \
             tc.tile_pool(name="w8p", bufs=1) as w8p, \
             tc.tile_pool(name="cst", bufs=1) as cst, \
             tc.tile_pool(name="ps", bufs=3, space="PSUM") as pp, \
             tc.tile_pool(name="pl", bufs=2, space="PSUM") as pl, \
             tc.tile_pool(name="dup", bufs=3) as dup, \
             tc.tile_pool(name="dmp", bufs=3) as dmp, \
             tc.tile_pool(name="dcp", bufs=2) as dcp, \
             tc.tile_pool(name="ep", bufs=2) as ep, \
             tc.tile_pool(name="e2p", bufs=2) as e2p, \
             tc.tile_pool(name="ob", bufs=8) as obp:

            x8 = x8p.tile([128, KC, 2, N], f8, tag="x8")
            for c in range(KC):
                nc.sync.dma_start(x8[:, c], x8_d[c])
            xbf = xbp.tile([128, KT, N], bf16, tag="xbf")
            for k in range(KT):
                nc.sync.dma_start(xbf[:, k], xbf_d[k])
            xts = xsp.tile([128, RT, D_IN], f32, tag="xres")
            for rt in range(RT):
                nc.sync.dma_start(xts[:, rt], x_d[rt])
            w8 = w8p.tile([128, HPC, TT, KC, 2, 512], f8, tag="w8")
            for h in range(HPC):
                nc.sync.dma_start(w8[:, h], w8_d[h])
            srt = cst.tile([128, HPC * D_H], bf16, tag="sr")
            nc.sync.dma_start(srt[:], sr_d[:])
            gtt = cst.tile([128, KT * HPC], bf16, tag="gt")
            nc.sync.dma_start(gtt[:], gt_d[:])
            b2r = cst.tile([128, HPC], f32, tag="b2r")
            nc.sync.dma_start(b2r[:], b2_d[:])

            for rt in range(RT):
                rs = rt * 128
                # x part of the 4 output blocks first, so ACT overlaps the GEMM
                obs = []
                for h in range(HPC):
                    ob = obp.tile([128, 513], f32, tag="ob")
                    nc.scalar.activation(ob[:, 0:512], xts[:, rt], copyf)
                    obs.append(ob)
                # linear term, all 4 heads batched
                plin = pl.tile([128, HPC], f32, tag="plin")
                for k in range(KT):
                    nc.tensor.matmul(
                        plin[:],
                        lhsT=xbf[:, k, rs:rs + 128],
                        rhs=gtt[:, k * HPC:(k + 1) * HPC],
                        start=(k == 0),
                        stop=(k == KT - 1),
                    )
                # fp8 GEMM + abs-accumulate, per (head, 2-bank group)
                dcol = dcp.tile([128, 2, HPC], f32, tag="dc")
                for h in range(HPC):
                    for g in range(2):
                        ps2 = pp.tile([128, 1024], f32, tag="ps")
                        for tl in range(2):
                            t = 2 * g + tl
                            for c in range(KC):
                                nc.tensor.matmul(
                                    ps2[:, tl * 512:(tl + 1) * 512],
                                    lhsT=x8[:, c, :, rs:rs + 128],
                                    rhs=w8[:, h, t, c],
                                    start=(c == 0),
                                    stop=(c == KC - 1),
                                    perf_mode=DR,
                                )
                        du = dup.tile([128, 1024], bf16, tag="du")
                        s0 = h * D_H + g * 1024
                        nc.scalar.activation(du[:], ps2[:], absf)
                        dm = dmp.tile([128, 1024], bf16, tag="dm")
                        nc.vector.scalar_tensor_tensor(
                            out=dm[:], in0=du[:], scalar=1.0,
                            in1=srt[:, s0:s0 + 1024],
                            op0=mult, op1=mult,
                            accum_out=dcol[:, g, h:h + 1],
                        )
                # combine: e = dcol_g0 + dcol_g1 ; e2 = plin + e
                e = ep.tile([128, HPC], f32, tag="e")
                nc.vector.scalar_tensor_tensor(
                    out=e[:], in0=dcol[:, 0], scalar=1.0,
                    in1=dcol[:, 1], op0=mult, op1=add,
                )
                e2 = e2p.tile([128, HPC], f32, tag="e2")
                nc.vector.scalar_tensor_tensor(
                    out=e2[:], in0=plin[:], scalar=1.0,
                    in1=e[:], op0=mult, op1=add,
                )
                for h in range(HPC):
                    nc.scalar.activation(
                        obs[h][:, 512:513], e2[:, h:h + 1], ident,
                        bias=b2r[:, h:h + 1], scale=1.0,
                    )
                    nc.sync.dma_start(
                        out_d[rs:rs + 128, h * 513:(h + 1) * 513], obs[h][:]
                    )

    nc.compile()
    return nc


def _get_program():
    global _PROG
    if _PROG is None:
        _PROG = _build_program()
    return _PROG


def kernel(x, W1, b1, W2, b2):
    import ml_dtypes
    from concourse.bass_utils import run_bass_kernel_spmd

    x = np.asarray(x, dtype=np.float32)
    W1 = np.asarray(W1, dtype=np.float32)
    b1 = np.asarray(b1, dtype=np.float32)
    W2 = np.asarray(W2, dtype=np.float32)
    b2 = np.asarray(b2, dtype=np.float32)

    nc = _get_program()

    bf = ml_dtypes.bfloat16
    e4 = ml_dtypes.float8_e4m3fn

    xT = np.ascontiguousarray(x.T)                       # [512, 4096]
    x8dr = np.ascontiguousarray(
        xT.astype(e4).reshape(KC, 2, 128, N).transpose(0, 2, 1, 3)
    )                                                    # [KC, 128, 2, N] fp8
    xbf = np.ascontiguousarray(xT.astype(bf).reshape(KT, 128, N))
    xres = np.ascontiguousarray(x.reshape(RT, 128, D_IN))

    in_maps = []
    for cre in range(NCORES):
        hs = slice(HPC * cre, HPC * (cre + 1))
        W1c = W1[hs]                                     # [HPC, 512, 2048]
        w2c = W2[hs]                                     # [HPC, 2048]
        aw = np.abs(w2c)
        # folded, scaled weights -> fp8 bytes, DR layout [H, p, t, c, i, j]
        wf = (SW * W1c * aw[:, None, :]).astype(e4)      # [HPC, 512, 2048]
        w8c = np.ascontiguousarray(
            wf.reshape(HPC, KC, 2, 128, TT, 512).transpose(0, 3, 4, 1, 2, 5)
        ).reshape(HPC, 128, TT * KC * 2 * 512)
        srow = np.broadcast_to(
            (np.sign(w2c) / (2.0 * SW)).reshape(1, HPC * D_H).astype(bf),
            (128, HPC * D_H))
        G = 0.5 * np.einsum("hdf,hf->dh", W1c.astype(np.float64),
                            w2c.astype(np.float64))      # [512, HPC]
        gt = np.ascontiguousarray(
            G.astype(np.float32).astype(bf).reshape(KT, 128, HPC)
            .transpose(1, 0, 2)).reshape(128, KT * HPC)
        gt = np.ascontiguousarray(gt)
        b2eff = b2[hs].astype(np.float64) + 0.5 * np.einsum(
            "hf,hf->h", b1[hs].astype(np.float64), w2c.astype(np.float64))
        b2rc = np.broadcast_to(
            b2eff.astype(np.float32).reshape(1, HPC), (128, HPC))
        in_maps.append({
            "x8": x8dr,
            "xbf": xbf,
            "x": xres,
            "w8": w8c,
            "srow": np.ascontiguousarray(srow),
            "gt": gt,
            "b2r": np.ascontiguousarray(b2rc),
        })

    import os
    trace = os.environ.get("BASS_KERNEL_TRACE") == "1"
    if trace:
        import sys
        sys.path.insert(0, "/tmp")
        try:
            import axon_shim
            axon_shim.install()
        except Exception:
            trace = False
    res = run_bass_kernel_spmd(nc, in_maps, list(range(NCORES)), trace=trace)
    kernel.last_result = res

    return np.concatenate([res.results[c]["out"] for c in range(NCORES)], axis=1)
